# revision 11
# baseline (speedup 1.0000x reference)
"""Trainium2 Bass kernel for MinimalKAN forward (nn_MinimalKAN_Normalized).

Math:
  a = sigmoid(alpha)
  out = (1-a) * (x @ W.T + b) + (a/sqrt(I)) * (x @ C0 + x^2 @ C1 + x^3 @ C2)

Folding the alpha blend into the weights on the host gives exactly
  out = x @ A + x^2 @ B + x^3 @ C + b_eff
with A = (1-a) W.T + s C0, B = s C1, C = s C2, b_eff = (1-a) b, s = a/sqrt(I).

Device strategy (data-parallel over batch, 8 cores), per core shard 4096 rows:
  The contraction index i sits on SBUF partitions for the TensorEngine, so the
  kernel consumes x^T (host-transposed, fp16).  Per 512-row batch group:
    - DMA x^T group [128, 4, 512] fp16 (SP queue)
    - ACT: x2 = Square(x/sqrt(8)) -> x^2/8 in fp8e4
    - DVE: x3 = x2 * x -> x^3/8 in fp8e4
    - per 128-row tile, one PSUM bank accumulates:
        4 fp16 matmuls   (xT k-slice [128,128]  @ A'[128,512])
        4 fp8e4 DoubleRow matmuls (2 k-slices each: [128,2,128] @ [128,2,512])
      DoubleRow runs at 0.5 cyc/row vs fp16's 1.0 -> KAN GEMMs cost half.
    - DVE scalar_tensor_tensor fuses the 1/SCALE rescale + bias add into the
      PSUM->SBUF eviction (bf16 out), out DMA on the ACT queue.
  Weight scales: A' = A*S (fp16), B' = B*8S, C' = C*8S (fp8e4, clipped to
  +-240); basis carries x^2/8, x^3/8 so fp8 stays well inside e4m3 range
  (x^3 max ~185 would be too close to 240).  PSUM holds S*(out - b_eff).
Output is written bf16 (halves the out DMA; |out|<~2 so the absolute error
stays ~2^-9*max) and upcast to f32 on the host.
"""

import os
import numpy as np

import concourse.bass as bass
from concourse import bacc
import concourse.mybir as mybir
import concourse.tile as tile
from concourse.bass_utils import run_bass_kernel_spmd

N_CORES = 8
B, I, O = 32768, 512, 512
BS = B // N_CORES          # rows per core
P = 128
N_TILES = BS // P          # 32 tiles per core
KS = I // P                # 4 contraction slices per basis
G = 4                      # tiles per group
GB = G * P                 # batch rows per group
N_GROUPS = N_TILES // G

SCALE = 4096.0             # unified PSUM scale (weights premultiplied)
BSCALE = 8.0               # basis carries x^k/8 for the fp8 terms

_OUT_F32 = os.environ.get("KAN_OUT_F32", "0") == "1"
_FP8 = os.environ.get("KAN_FP8", "1") == "1"
# all-fp8 linear path: x = x8 + xlo split on host, A = A_hi + A_lo split on
# host, linear term = x8@A_hi + xlo@A_hi + x8@A_lo (all DoubleRow fp8);
# basis is derived from x8 on device.
_ALL8 = os.environ.get("KAN_ALL8", "0") == "1"


def _build(repeat: int = 1, out_f32: bool = _OUT_F32, fp8: bool = _FP8,
           all8: bool = _ALL8) -> bass.Bass:
    f16 = mybir.dt.float16
    f32 = mybir.dt.float32
    bf16 = mybir.dt.bfloat16
    f8 = mybir.dt.float8e4
    sq = mybir.ActivationFunctionType.Square
    DR = mybir.MatmulPerfMode.DoubleRow
    out_dt = f32 if out_f32 else bf16
    x_dt = f8 if all8 else f16
    xks = 2 * KS if all8 else KS

    nc = bacc.Bacc("TRN2", target_bir_lowering=False, debug=False,
                   num_devices=N_CORES)

    x_d = nc.dram_tensor("xt", [xks * P, BS], x_dt, kind="ExternalInput")
    x_r = x_d.rearrange("(ks p) b -> p ks b", p=P)
    wa_d = nc.dram_tensor("wa", [xks * P, O], x_dt, kind="ExternalInput")
    wa_r = wa_d.rearrange("(ks p) o -> p ks o", p=P)
    if fp8:
        w8_d = nc.dram_tensor("w8", [2 * I, O], f8, kind="ExternalInput")
    else:
        w8_d = nc.dram_tensor("w8", [2 * I, O], f16, kind="ExternalInput")
    w8_r = w8_d.rearrange("(ks p) o -> p ks o", p=P)
    b_d = nc.dram_tensor("bias", [P, O], f32, kind="ExternalInput")
    o_d = nc.dram_tensor("out", [BS, O], out_dt, kind="ExternalOutput")
    o_g = o_d.rearrange("(g a p) k -> g p a k", a=G, p=P)
    o_1 = o_d.rearrange("(t p) k -> t p k", p=P)

    # single-tile units at the edges ramp the pipeline in/out quickly
    # (first matmul needs only a 128-col x slice; last eviction+DMA is small);
    # 4-tile units in the middle amortize instruction overheads.
    units = [(t, 1) for t in range(4)]
    units += [(t, G) for t in range(4, N_TILES - 4, G)]
    units += [(t, 1) for t in range(N_TILES - 4, N_TILES)]

    with tile.TileContext(nc) as tc:
        with (
            tc.tile_pool(name="const", bufs=1) as const,
            tc.tile_pool(name="xt", bufs=3) as xt,
            tc.tile_pool(name="x8", bufs=3) as x8,
            tc.tile_pool(name="outp", bufs=3) as outp,
            tc.tile_pool(name="psum_o", bufs=6, space="PSUM") as psum_o,
        ):
            # weights on the gpsimd SWDGE queue so the SP queue starts with group 0's x
            wa_sb = const.tile([P, KS, O], f16)
            nc.gpsimd.dma_start(wa_sb[:], wa_r[:, :, :])
            w8_sb = const.tile([P, 2 * KS, O], f8 if fp8 else f16)
            nc.gpsimd.dma_start(w8_sb[:], w8_r[:, :, :])
            bsb = const.tile([P, O], f32)
            nc.gpsimd.dma_start(bsb[:], b_d[:, :])

            for t0, gu in [u for _ in range(repeat) for u in units]:
                ub = gu * P
                xT = xt.tile([P, KS, ub], f16, tag="xT")
                nc.sync.dma_start(xT[:], x_r[:, :, t0 * P:t0 * P + ub])

                x2 = x8.tile([P, KS, ub], f8 if fp8 else f16, tag="x2")
                # Square(x * 1/sqrt(8)) = x^2/8
                nc.scalar.activation(x2[:], xT[:], sq,
                                     scale=1.0 / np.sqrt(BSCALE))
                x3 = x8.tile([P, KS, ub], f8 if fp8 else f16, tag="x3")
                # ramp-out singles: x^3 on gpsimd so it isn't queued behind
                # the DVE evictions of the previous groups
                mul_eng = (nc.gpsimd if gu == 1 and t0 >= N_TILES - 4
                           else nc.vector)
                mul_eng.tensor_mul(x3[:], x2[:], xT[:])

                o_sb = outp.tile([P, gu, O], out_dt, tag="o_sb")
                for j in range(gu):
                    js = slice(j * P, (j + 1) * P)
                    po = psum_o.tile([P, O], f32, tag="po", name="po")
                    for k in range(KS):
                        nc.tensor.matmul(po[:], xT[:, k, js], wa_sb[:, k, :],
                                         start=(k == 0), stop=False,
                                         skip_group_check=True)
                    if fp8:
                        for i2, XT in ((0, x2), (2, x2), (4, x3), (6, x3)):
                            ks2 = slice(i2 % 4, i2 % 4 + 2)
                            nc.tensor.matmul(po[:], XT[:, ks2, js],
                                             w8_sb[:, i2:i2 + 2, :],
                                             perf_mode=DR,
                                             start=False, stop=(i2 == 6),
                                             skip_group_check=True)
                    else:
                        for i2, XT in enumerate([x2] * 4 + [x3] * 4):
                            nc.tensor.matmul(po[:], XT[:, i2 % 4, js],
                                             w8_sb[:, i2, :],
                                             start=False, stop=(i2 == 7),
                                             skip_group_check=True)
                    nc.vector.scalar_tensor_tensor(
                        o_sb[:, j, :], po[:], 1.0 / SCALE, bsb[:],
                        mybir.AluOpType.mult, mybir.AluOpType.add)
                oq = (nc.sync if gu == 1 and t0 in (N_TILES - 2, N_TILES - 4)
                      else nc.scalar)
                if gu == 1:
                    oq.dma_start(o_1[t0], o_sb[:, 0, :])
                else:
                    oq.dma_start(o_g[t0 // G], o_sb[:])

    nc.compile()
    return nc


_NC_CACHE: dict[tuple, bass.Bass] = {}


def _get_nc() -> bass.Bass:
    key = (_OUT_F32, _FP8)
    nc = _NC_CACHE.get(key)
    if nc is None:
        nc = _build()
        _NC_CACHE[key] = nc
    return nc


def _fold_weights(coeffs, W, b, alpha):
    import ml_dtypes

    a = 1.0 / (1.0 + np.exp(-np.float64(alpha)))
    s = a / np.sqrt(np.float64(I))
    A = (1.0 - a) * W.astype(np.float64).T + s * coeffs[:, :, 0].astype(np.float64)
    Bm = s * coeffs[:, :, 1].astype(np.float64)
    Cm = s * coeffs[:, :, 2].astype(np.float64)
    wa = np.ascontiguousarray((A * SCALE).astype(np.float16))
    w8f = np.concatenate([Bm, Cm], axis=0) * (BSCALE * SCALE)
    if _FP8:
        w8 = np.ascontiguousarray(
            np.clip(w8f, -240.0, 240.0).astype(ml_dtypes.float8_e4m3))
    else:
        w8 = np.ascontiguousarray(w8f.astype(np.float16))
    b_eff = ((1.0 - a) * b.astype(np.float64)).astype(np.float32)
    bias_rep = np.ascontiguousarray(
        np.broadcast_to(b_eff[None, :], (P, O)).astype(np.float32))
    return wa, w8, bias_rep


def _make_in_maps(x, coeffs, W, b, alpha):
    wa, w8, bias_rep = _fold_weights(coeffs, W, b, alpha)
    x = np.asarray(x, dtype=np.float32)
    in_maps = []
    for c in range(N_CORES):
        shard = x[c * BS:(c + 1) * BS]
        in_maps.append({
            "wa": wa, "w8": w8, "bias": bias_rep,
            "xt": np.ascontiguousarray(shard.T.astype(np.float16)),
        })
    return in_maps


def _run(x, coeffs, W, b, alpha, trace=False):
    nc = _get_nc()
    in_maps = _make_in_maps(x, coeffs, W, b, alpha)
    res = run_bass_kernel_spmd(nc, in_maps, core_ids=list(range(N_CORES)),
                               trace=trace)
    out = np.concatenate(
        [np.asarray(r["out"], dtype=np.float32) for r in res.results], axis=0)
    return out, res


def kernel(x, coeffs, W, b, alpha):
    out, _ = _run(x, coeffs, W, b, alpha, trace=False)
    return out


# revision 23
# speedup vs baseline: 1.1418x; 1.1418x over previous
"""Trainium2 Bass kernel for MinimalKAN forward (nn_MinimalKAN_Normalized).

Math:
  a = sigmoid(alpha)
  out = (1-a) * (x @ W.T + b) + (a/sqrt(I)) * (x @ C0 + x^2 @ C1 + x^3 @ C2)

Folding the alpha blend into the weights on the host gives exactly
  out = x @ A + x^2 @ B + x^3 @ C + b_eff
with A = (1-a) W.T + s C0, B = s C1, C = s C2, b_eff = (1-a) b, s = a/sqrt(I).

Device strategy (data-parallel over batch, 8 cores), per core shard 4096 rows:
  The contraction index i sits on SBUF partitions for the TensorEngine, so the
  kernel consumes x^T (host-transposed, fp16).  Per 512-row batch group:
    - DMA x^T group [128, 4, 512] fp16 (SP queue)
    - ACT: x2 = Square(x/sqrt(8)) -> x^2/8 in fp8e4
    - DVE: x3 = x2 * x -> x^3/8 in fp8e4
    - per 128-row tile, one PSUM bank accumulates:
        4 fp16 matmuls   (xT k-slice [128,128]  @ A'[128,512])
        4 fp8e4 DoubleRow matmuls (2 k-slices each: [128,2,128] @ [128,2,512])
      DoubleRow runs at 0.5 cyc/row vs fp16's 1.0 -> KAN GEMMs cost half.
    - DVE scalar_tensor_tensor fuses the 1/SCALE rescale + bias add into the
      PSUM->SBUF eviction (bf16 out), out DMA on the ACT queue.
  Weight scales: A' = A*S (fp16), B' = B*8S, C' = C*8S (fp8e4, clipped to
  +-240); basis carries x^2/8, x^3/8 so fp8 stays well inside e4m3 range
  (x^3 max ~185 would be too close to 240).  PSUM holds S*(out - b_eff).
Output is written bf16 (halves the out DMA; |out|<~2 so the absolute error
stays ~2^-9*max) and upcast to f32 on the host.
"""

import os
import numpy as np

import concourse.bass as bass
from concourse import bacc
import concourse.mybir as mybir
import concourse.tile as tile
from concourse.bass_utils import run_bass_kernel_spmd

N_CORES = 8
B, I, O = 32768, 512, 512
BS = B // N_CORES          # rows per core
P = 128
N_TILES = BS // P          # 32 tiles per core
KS = I // P                # 4 contraction slices per basis
G = 4                      # tiles per group
GB = G * P                 # batch rows per group
N_GROUPS = N_TILES // G

SCALE = 4096.0             # unified PSUM scale (weights premultiplied)
BSCALE = 8.0               # basis carries x^k/8 for the fp8 terms

_OUT_F32 = os.environ.get("KAN_OUT_F32", "0") == "1"
_FP8 = os.environ.get("KAN_FP8", "1") == "1"
# all-fp8 linear path: x = x8 + xlo split on host, A = A_hi + A_lo split on
# host, linear term = x8@A_hi + xlo@A_hi + x8@A_lo (all DoubleRow fp8);
# basis is derived from x8 on device.
_ALL8 = os.environ.get("KAN_ALL8", "0") == "1"
# phase-batch the fp16 and fp8-DR matmuls of a unit (all fp16 for every tile,
# then all DR) to amortize the PE mode-switch penalty over the whole unit.
_PHASED = os.environ.get("KAN_PHASED", "1") == "1"
_RAMP = os.environ.get("KAN_RAMP", "1") == "1"
# pair two 4-tile units per PE phase pair (fp16 a, fp16 b, DR a, DR b) using
# all 8 PSUM banks: 2 mode switches per 8 tiles instead of per 4.
_PAIR = os.environ.get("KAN_PAIR", "0") == "1"


def _build(repeat: int = 1, out_f32: bool = _OUT_F32, fp8: bool = _FP8,
           all8: bool = _ALL8) -> bass.Bass:
    f16 = mybir.dt.float16
    f32 = mybir.dt.float32
    bf16 = mybir.dt.bfloat16
    f8 = mybir.dt.float8e4
    sq = mybir.ActivationFunctionType.Square
    DR = mybir.MatmulPerfMode.DoubleRow
    out_dt = f32 if out_f32 else bf16
    x_dt = f8 if all8 else f16
    xks = 2 * KS if all8 else KS

    nc = bacc.Bacc("TRN2", target_bir_lowering=False, debug=False,
                   num_devices=N_CORES)

    x_d = nc.dram_tensor("xt", [xks * P, BS], x_dt, kind="ExternalInput")
    x_r = x_d.rearrange("(ks p) b -> p ks b", p=P)
    wa_d = nc.dram_tensor("wa", [xks * P, O], x_dt, kind="ExternalInput")
    wa_r = wa_d.rearrange("(ks p) o -> p ks o", p=P)
    if fp8:
        w8_d = nc.dram_tensor("w8", [2 * I, O], f8, kind="ExternalInput")
    else:
        w8_d = nc.dram_tensor("w8", [2 * I, O], f16, kind="ExternalInput")
    w8_r = w8_d.rearrange("(ks p) o -> p ks o", p=P)
    b_d = nc.dram_tensor("bias", [P, O], f32, kind="ExternalInput")
    o_d = nc.dram_tensor("out", [BS, O], out_dt, kind="ExternalOutput")
    o_g = o_d.rearrange("(g a p) k -> g p a k", a=G, p=P)
    o_1 = o_d.rearrange("(t p) k -> t p k", p=P)

    # single-tile units at the edges ramp the pipeline in/out quickly
    # (first matmul needs only a 128-col x slice; last eviction+DMA is small);
    # 4-tile units in the middle amortize instruction overheads.  In phased
    # mode singles are counterproductive (each pays 2 PE mode switches).
    if _RAMP and not _PHASED:
        units = [(t, 1) for t in range(4)]
        units += [(t, G) for t in range(4, N_TILES - 4, G)]
        units += [(t, 1) for t in range(N_TILES - 4, N_TILES)]
    else:
        units = [(t, G) for t in range(0, N_TILES, G)]

    with tile.TileContext(nc) as tc:
        with (
            tc.tile_pool(name="const", bufs=1) as const,
            tc.tile_pool(name="xt", bufs=3) as xt,
            tc.tile_pool(name="x8", bufs=3) as x8,
            tc.tile_pool(name="outp", bufs=3) as outp,
            tc.tile_pool(name="psum_o", bufs=8 if _PAIR else 6,
                         space="PSUM") as psum_o,
        ):
            # weights on the gpsimd SWDGE queue so the SP queue starts with group 0's x
            wa_sb = const.tile([P, xks, O], x_dt)
            nc.gpsimd.dma_start(wa_sb[:], wa_r[:, :, :])
            w8_sb = const.tile([P, 2 * KS, O], f8 if fp8 else f16)
            nc.gpsimd.dma_start(w8_sb[:], w8_r[:, :, :])
            bsb = const.tile([P, O], f32)
            nc.gpsimd.dma_start(bsb[:], b_d[:, :])

            if _PAIR and not all8:
                pairs = [(units[i], units[i + 1])
                         for i in range(0, len(units), 2)]
                for ua, ub_ in [p for _ in range(repeat) for p in pairs]:
                    members = []
                    for t0, gu in (ua, ub_):
                        w = gu * P
                        xT = xt.tile([P, KS, w], f16, tag="xT")
                        nc.sync.dma_start(
                            xT[:], x_r[:, :, t0 * P:t0 * P + w])
                        x2 = x8.tile([P, KS, w], f8, tag="x2")
                        nc.scalar.activation(x2[:], xT[:], sq,
                                             scale=1.0 / np.sqrt(BSCALE))
                        x3 = x8.tile([P, KS, w], f8, tag="x3")
                        nc.vector.tensor_mul(x3[:], x2[:], xT[:])
                        o_sb = outp.tile([P, gu, O], out_dt, tag="o_sb")
                        members.append((t0, gu, xT, x2, x3, o_sb))
                    pos_all = []
                    for t0, gu, xT, x2, x3, o_sb in members:
                        pos = [psum_o.tile([P, O], f32, tag="po", name="po")
                               for _ in range(gu)]
                        pos_all.append(pos)
                        for k in range(KS):
                            for j in range(gu):
                                js = slice(j * P, (j + 1) * P)
                                nc.tensor.matmul(
                                    pos[j][:], xT[:, k, js], wa_sb[:, k, :],
                                    start=(k == 0), stop=False,
                                    skip_group_check=True)
                    for pos, (t0, gu, xT, x2, x3, o_sb) in zip(pos_all,
                                                               members):
                        for i2, XT in ((0, x2), (2, x2), (4, x3), (6, x3)):
                            ks2 = slice(i2 % 4, i2 % 4 + 2)
                            for j in range(gu):
                                js = slice(j * P, (j + 1) * P)
                                nc.tensor.matmul(
                                    pos[j][:], XT[:, ks2, js],
                                    w8_sb[:, i2:i2 + 2, :], perf_mode=DR,
                                    start=False, stop=(i2 == 6),
                                    skip_group_check=True)
                        for j in range(gu):
                            nc.vector.scalar_tensor_tensor(
                                o_sb[:, j, :], pos[j][:], 1.0 / SCALE,
                                bsb[:], mybir.AluOpType.mult,
                                mybir.AluOpType.add)
                        nc.scalar.dma_start(o_g[t0 // G], o_sb[:])
                units_left = []
            else:
                units_left = [u for _ in range(repeat) for u in units]

            for t0, gu in units_left:
                ub = gu * P
                xT = xt.tile([P, xks, ub], x_dt, tag="xT")
                nc.sync.dma_start(xT[:], x_r[:, :, t0 * P:t0 * P + ub])
                xm = xT[:, 0:KS, :]   # x8 part when all8, whole x otherwise

                x2 = x8.tile([P, KS, ub], f8 if fp8 else f16, tag="x2")
                # Square(x * 1/sqrt(8)) = x^2/8
                nc.scalar.activation(x2[:], xm, sq,
                                     scale=1.0 / np.sqrt(BSCALE))
                x3 = x8.tile([P, KS, ub], f8 if fp8 else f16, tag="x3")
                # ramp-out singles: x^3 on gpsimd so it isn't queued behind
                # the DVE evictions of the previous groups; in all8 mode also
                # alternate steady groups onto gpsimd to keep DVE under PE
                mul_eng = nc.vector
                if gu == 1 and t0 >= N_TILES - 4:
                    mul_eng = nc.gpsimd
                elif all8 and gu == G and (t0 // G) % 2 == 0:
                    mul_eng = nc.gpsimd
                mul_eng.tensor_mul(x3[:], x2[:], xm)

                o_sb = outp.tile([P, gu, O], out_dt, tag="o_sb")
                if _PHASED and not all8:
                    # all fp16 matmuls of the unit first, then all fp8-DR:
                    # 2 PE mode transitions per unit instead of 2 per tile
                    pos = [psum_o.tile([P, O], f32, tag="po", name="po")
                           for _ in range(gu)]
                    for k in range(KS):
                        for j in range(gu):
                            js = slice(j * P, (j + 1) * P)
                            nc.tensor.matmul(pos[j][:], xT[:, k, js],
                                             wa_sb[:, k, :],
                                             start=(k == 0), stop=False,
                                             skip_group_check=True)
                    for i2, XT in ((0, x2), (2, x2), (4, x3), (6, x3)):
                        ks2 = slice(i2 % 4, i2 % 4 + 2)
                        for j in range(gu):
                            js = slice(j * P, (j + 1) * P)
                            nc.tensor.matmul(pos[j][:], XT[:, ks2, js],
                                             w8_sb[:, i2:i2 + 2, :],
                                             perf_mode=DR,
                                             start=False, stop=(i2 == 6),
                                             skip_group_check=True)
                    for j in range(gu):
                        nc.vector.scalar_tensor_tensor(
                            o_sb[:, j, :], pos[j][:], 1.0 / SCALE, bsb[:],
                            mybir.AluOpType.mult, mybir.AluOpType.add)
                    if gu == 1:
                        nc.scalar.dma_start(o_1[t0], o_sb[:, 0, :])
                    else:
                        nc.scalar.dma_start(o_g[t0 // G], o_sb[:])
                    continue
                for j in range(gu):
                    js = slice(j * P, (j + 1) * P)
                    po = psum_o.tile([P, O], f32, tag="po", name="po")
                    if all8:
                        mms = [(xT, 0, wa_sb, 0), (xT, 2, wa_sb, 2),
                               (xT, 4, wa_sb, 0), (xT, 6, wa_sb, 2),
                               (xT, 0, wa_sb, 4), (xT, 2, wa_sb, 6),
                               (x2, 0, w8_sb, 0), (x2, 2, w8_sb, 2),
                               (x3, 0, w8_sb, 4), (x3, 2, w8_sb, 6)]
                        for i2, (XT, xk, WS, wk) in enumerate(mms):
                            nc.tensor.matmul(po[:], XT[:, xk:xk + 2, js],
                                             WS[:, wk:wk + 2, :],
                                             perf_mode=DR,
                                             start=(i2 == 0),
                                             stop=(i2 == len(mms) - 1),
                                             skip_group_check=True)
                    else:
                        for k in range(KS):
                            nc.tensor.matmul(po[:], xT[:, k, js],
                                             wa_sb[:, k, :],
                                             start=(k == 0), stop=False,
                                             skip_group_check=True)
                        if fp8:
                            for i2, XT in ((0, x2), (2, x2), (4, x3), (6, x3)):
                                ks2 = slice(i2 % 4, i2 % 4 + 2)
                                nc.tensor.matmul(po[:], XT[:, ks2, js],
                                                 w8_sb[:, i2:i2 + 2, :],
                                                 perf_mode=DR,
                                                 start=False, stop=(i2 == 6),
                                                 skip_group_check=True)
                        else:
                            for i2, XT in enumerate([x2] * 4 + [x3] * 4):
                                nc.tensor.matmul(po[:], XT[:, i2 % 4, js],
                                                 w8_sb[:, i2, :],
                                                 start=False, stop=(i2 == 7),
                                                 skip_group_check=True)
                    nc.vector.scalar_tensor_tensor(
                        o_sb[:, j, :], po[:], 1.0 / SCALE, bsb[:],
                        mybir.AluOpType.mult, mybir.AluOpType.add)
                oq = (nc.sync if gu == 1 and t0 in (N_TILES - 2, N_TILES - 4)
                      else nc.scalar)
                if gu == 1:
                    oq.dma_start(o_1[t0], o_sb[:, 0, :])
                else:
                    oq.dma_start(o_g[t0 // G], o_sb[:])

    nc.compile()
    return nc


_NC_CACHE: dict[tuple, bass.Bass] = {}


def _get_nc() -> bass.Bass:
    key = (_OUT_F32, _FP8)
    nc = _NC_CACHE.get(key)
    if nc is None:
        nc = _build()
        _NC_CACHE[key] = nc
    return nc


def _fold_weights(coeffs, W, b, alpha):
    import ml_dtypes

    a = 1.0 / (1.0 + np.exp(-np.float64(alpha)))
    s = a / np.sqrt(np.float64(I))
    A = (1.0 - a) * W.astype(np.float64).T + s * coeffs[:, :, 0].astype(np.float64)
    Bm = s * coeffs[:, :, 1].astype(np.float64)
    Cm = s * coeffs[:, :, 2].astype(np.float64)
    if _ALL8:
        asf = (A * SCALE).astype(np.float32)
        a_hi = asf.astype(ml_dtypes.float8_e4m3)
        a_lo = (asf - a_hi.astype(np.float32)).astype(ml_dtypes.float8_e4m3)
        wa = np.ascontiguousarray(np.concatenate([a_hi, a_lo], axis=0))
    else:
        wa = np.ascontiguousarray((A * SCALE).astype(np.float16))
    w8f = np.concatenate([Bm, Cm], axis=0) * (BSCALE * SCALE)
    if _FP8:
        w8 = np.ascontiguousarray(
            np.clip(w8f, -240.0, 240.0).astype(ml_dtypes.float8_e4m3))
    else:
        w8 = np.ascontiguousarray(w8f.astype(np.float16))
    b_eff = ((1.0 - a) * b.astype(np.float64)).astype(np.float32)
    bias_rep = np.ascontiguousarray(
        np.broadcast_to(b_eff[None, :], (P, O)).astype(np.float32))
    return wa, w8, bias_rep


def _make_in_maps(x, coeffs, W, b, alpha):
    wa, w8, bias_rep = _fold_weights(coeffs, W, b, alpha)
    x = np.asarray(x, dtype=np.float32)
    in_maps = []
    for c in range(N_CORES):
        shard = x[c * BS:(c + 1) * BS]
        if _ALL8:
            import ml_dtypes

            xt_f = shard.T.astype(np.float32)
            x_hi = xt_f.astype(ml_dtypes.float8_e4m3)
            x_lo = (xt_f - x_hi.astype(np.float32)).astype(
                ml_dtypes.float8_e4m3)
            xt_v = np.ascontiguousarray(np.concatenate([x_hi, x_lo], axis=0))
        else:
            xt_v = np.ascontiguousarray(shard.T.astype(np.float16))
        in_maps.append({
            "wa": wa, "w8": w8, "bias": bias_rep,
            "xt": xt_v,
        })
    return in_maps


def _run(x, coeffs, W, b, alpha, trace=False):
    nc = _get_nc()
    in_maps = _make_in_maps(x, coeffs, W, b, alpha)
    res = run_bass_kernel_spmd(nc, in_maps, core_ids=list(range(N_CORES)),
                               trace=trace)
    out = np.concatenate(
        [np.asarray(r["out"], dtype=np.float32) for r in res.results], axis=0)
    return out, res


def kernel(x, coeffs, W, b, alpha):
    out, _ = _run(x, coeffs, W, b, alpha, trace=False)
    return out


# revision 24
# speedup vs baseline: 1.7433x; 1.5267x over previous
"""Trainium2 Bass kernel for MinimalKAN forward (nn_MinimalKAN_Normalized).

Math:
  a = sigmoid(alpha)
  out = (1-a) * (x @ W.T + b) + (a/sqrt(I)) * (x @ C0 + x^2 @ C1 + x^3 @ C2)

Folding the alpha blend into the weights on the host gives exactly
  out = x @ A + x^2 @ B + x^3 @ C + b_eff
with A = (1-a) W.T + s C0, B = s C1, C = s C2, b_eff = (1-a) b, s = a/sqrt(I).

Device strategy (data-parallel over batch, 8 cores), per core shard 4096 rows:
  The contraction index i sits on SBUF partitions for the TensorEngine, so the
  kernel consumes x^T (host-transposed, fp16).  Per 512-row batch group:
    - DMA x^T group [128, 4, 512] fp16 (SP queue)
    - ACT: x2 = Square(x/sqrt(8)) -> x^2/8 in fp8e4
    - DVE: x3 = x2 * x -> x^3/8 in fp8e4
    - per 128-row tile, one PSUM bank accumulates:
        4 fp16 matmuls   (xT k-slice [128,128]  @ A'[128,512])
        4 fp8e4 DoubleRow matmuls (2 k-slices each: [128,2,128] @ [128,2,512])
      DoubleRow runs at 0.5 cyc/row vs fp16's 1.0 -> KAN GEMMs cost half.
    - DVE scalar_tensor_tensor fuses the 1/SCALE rescale + bias add into the
      PSUM->SBUF eviction (bf16 out), out DMA on the ACT queue.
  Weight scales: A' = A*S (fp16), B' = B*8S, C' = C*8S (fp8e4, clipped to
  +-240); basis carries x^2/8, x^3/8 so fp8 stays well inside e4m3 range
  (x^3 max ~185 would be too close to 240).  PSUM holds S*(out - b_eff).
Output is written bf16 (halves the out DMA; |out|<~2 so the absolute error
stays ~2^-9*max) and upcast to f32 on the host.
"""

import os
import numpy as np

import concourse.bass as bass
from concourse import bacc
import concourse.mybir as mybir
import concourse.tile as tile
from concourse.bass_utils import run_bass_kernel_spmd

N_CORES = 8
B, I, O = 32768, 512, 512
BS = B // N_CORES          # rows per core
P = 128
N_TILES = BS // P          # 32 tiles per core
KS = I // P                # 4 contraction slices per basis
G = 4                      # tiles per group
GB = G * P                 # batch rows per group
N_GROUPS = N_TILES // G

SCALE = 4096.0             # unified PSUM scale (weights premultiplied)
BSCALE = 8.0               # basis carries x^k/8 for the fp8 terms

_OUT_F32 = os.environ.get("KAN_OUT_F32", "0") == "1"
_FP8 = os.environ.get("KAN_FP8", "1") == "1"
# all-fp8 linear path: x = x8 + xlo split on host, A = A_hi + A_lo split on
# host, linear term = x8@A_hi + xlo@A_hi + x8@A_lo (all DoubleRow fp8);
# basis is derived from x8 on device.
_ALL8 = os.environ.get("KAN_ALL8", "0") == "1"
# phase-batch the fp16 and fp8-DR matmuls of a unit (all fp16 for every tile,
# then all DR) to amortize the PE mode-switch penalty over the whole unit.
_PHASED = os.environ.get("KAN_PHASED", "1") == "1"
_RAMP = os.environ.get("KAN_RAMP", "1") == "1"
# pair two 4-tile units per PE phase pair (fp16 a, fp16 b, DR a, DR b) using
# all 8 PSUM banks: 2 mode switches per 8 tiles instead of per 4.
_PAIR = os.environ.get("KAN_PAIR", "0") == "1"


def _build(repeat: int = 1, out_f32: bool = _OUT_F32, fp8: bool = _FP8,
           all8: bool = _ALL8) -> bass.Bass:
    f16 = mybir.dt.float16
    f32 = mybir.dt.float32
    bf16 = mybir.dt.bfloat16
    f8 = mybir.dt.float8e4
    sq = mybir.ActivationFunctionType.Square
    DR = mybir.MatmulPerfMode.DoubleRow
    out_dt = f32 if out_f32 else bf16
    x_dt = f8 if all8 else f16
    xks = 2 * KS if all8 else KS

    nc = bacc.Bacc("TRN2", target_bir_lowering=False, debug=False,
                   num_devices=N_CORES)

    x_d = nc.dram_tensor("xt", [xks * P, BS], x_dt, kind="ExternalInput")
    x_r = x_d.rearrange("(ks p) b -> p ks b", p=P)
    wa_d = nc.dram_tensor("wa", [xks * P, O], x_dt, kind="ExternalInput")
    wa_r = wa_d.rearrange("(ks p) o -> p ks o", p=P)
    if fp8:
        w8_d = nc.dram_tensor("w8", [2 * I, O], f8, kind="ExternalInput")
    else:
        w8_d = nc.dram_tensor("w8", [2 * I, O], f16, kind="ExternalInput")
    w8_r = w8_d.rearrange("(ks p) o -> p ks o", p=P)
    b_d = nc.dram_tensor("bias", [P, O], f32, kind="ExternalInput")
    o_d = nc.dram_tensor("out", [BS, O], out_dt, kind="ExternalOutput")
    o_g = o_d.rearrange("(g a p) k -> g p a k", a=G, p=P)
    o_1 = o_d.rearrange("(t p) k -> t p k", p=P)

    # single-tile units at the edges ramp the pipeline in/out quickly
    # (first matmul needs only a 128-col x slice; last eviction+DMA is small);
    # 4-tile units in the middle amortize instruction overheads.  In phased
    # mode singles are counterproductive (each pays 2 PE mode switches).
    if _RAMP and not _PHASED:
        units = [(t, 1) for t in range(4)]
        units += [(t, G) for t in range(4, N_TILES - 4, G)]
        units += [(t, 1) for t in range(N_TILES - 4, N_TILES)]
    else:
        units = [(t, G) for t in range(0, N_TILES, G)]

    with tile.TileContext(nc) as tc:
        with (
            tc.tile_pool(name="const", bufs=1) as const,
            tc.tile_pool(name="xt", bufs=3) as xt,
            tc.tile_pool(name="x8", bufs=3) as x8,
            tc.tile_pool(name="outp", bufs=3) as outp,
            tc.tile_pool(name="psum_o", bufs=8 if _PAIR else 6,
                         space="PSUM") as psum_o,
        ):
            # weights on the gpsimd SWDGE queue so the SP queue starts with group 0's x
            wa_sb = const.tile([P, xks, O], x_dt)
            nc.gpsimd.dma_start(wa_sb[:], wa_r[:, :, :])
            w8_sb = const.tile([P, 2 * KS, O], f8 if fp8 else f16)
            nc.gpsimd.dma_start(w8_sb[:], w8_r[:, :, :])
            bsb = const.tile([P, O], f32)
            nc.gpsimd.dma_start(bsb[:], b_d[:, :])

            if _PAIR and not all8:
                pairs = [(units[i], units[i + 1])
                         for i in range(0, len(units), 2)]
                for ua, ub_ in [p for _ in range(repeat) for p in pairs]:
                    members = []
                    for t0, gu in (ua, ub_):
                        w = gu * P
                        xT = xt.tile([P, KS, w], f16, tag="xT")
                        nc.sync.dma_start(
                            xT[:], x_r[:, :, t0 * P:t0 * P + w])
                        x2 = x8.tile([P, KS, w], f8, tag="x2")
                        nc.scalar.activation(x2[:], xT[:], sq,
                                             scale=1.0 / np.sqrt(BSCALE))
                        x3 = x8.tile([P, KS, w], f8, tag="x3")
                        nc.vector.tensor_mul(x3[:], x2[:], xT[:])
                        o_sb = outp.tile([P, gu, O], out_dt, tag="o_sb")
                        members.append((t0, gu, xT, x2, x3, o_sb))
                    pos_all = []
                    for t0, gu, xT, x2, x3, o_sb in members:
                        pos = [psum_o.tile([P, O], f32, tag="po", name="po")
                               for _ in range(gu)]
                        pos_all.append(pos)
                        for k in range(KS):
                            for j in range(gu):
                                js = slice(j * P, (j + 1) * P)
                                nc.tensor.matmul(
                                    pos[j][:], xT[:, k, js], wa_sb[:, k, :],
                                    start=(k == 0), stop=False,
                                    skip_group_check=True)
                    for pos, (t0, gu, xT, x2, x3, o_sb) in zip(pos_all,
                                                               members):
                        for i2, XT in ((0, x2), (2, x2), (4, x3), (6, x3)):
                            ks2 = slice(i2 % 4, i2 % 4 + 2)
                            for j in range(gu):
                                js = slice(j * P, (j + 1) * P)
                                nc.tensor.matmul(
                                    pos[j][:], XT[:, ks2, js],
                                    w8_sb[:, i2:i2 + 2, :], perf_mode=DR,
                                    start=False, stop=(i2 == 6),
                                    skip_group_check=True)
                        for j in range(gu):
                            nc.vector.scalar_tensor_tensor(
                                o_sb[:, j, :], pos[j][:], 1.0 / SCALE,
                                bsb[:], mybir.AluOpType.mult,
                                mybir.AluOpType.add)
                        nc.scalar.dma_start(o_g[t0 // G], o_sb[:])
                units_left = []
            else:
                units_left = [u for _ in range(repeat) for u in units]

            for t0, gu in units_left:
                ub = gu * P
                xT = xt.tile([P, xks, ub], x_dt, tag="xT")
                nc.sync.dma_start(xT[:], x_r[:, :, t0 * P:t0 * P + ub])
                xm = xT[:, 0:KS, :]   # x8 part when all8, whole x otherwise

                x2 = x8.tile([P, KS, ub], f8 if fp8 else f16, tag="x2")
                # Square(x * 1/sqrt(8)) = x^2/8
                nc.scalar.activation(x2[:], xm, sq,
                                     scale=1.0 / np.sqrt(BSCALE))
                x3 = x8.tile([P, KS, ub], f8 if fp8 else f16, tag="x3")
                # ramp-out singles: x^3 on gpsimd so it isn't queued behind
                # the DVE evictions of the previous groups; in all8 mode also
                # alternate steady groups onto gpsimd to keep DVE under PE
                mul_eng = nc.vector
                if gu == 1 and t0 >= N_TILES - 4:
                    mul_eng = nc.gpsimd
                elif all8 and gu == G and (t0 // G) % 2 == 0:
                    mul_eng = nc.gpsimd
                mul_eng.tensor_mul(x3[:], x2[:], xm)

                o_sb = outp.tile([P, gu, O], out_dt, tag="o_sb")
                if _PHASED and not all8:
                    # all fp16 matmuls of the unit first, then all fp8-DR:
                    # 2 PE mode transitions per unit instead of 2 per tile.
                    # j-outer: each PSUM bank gets its 4 matmuls back-to-back
                    # (KAN_JOUTER=0 interleaves banks every instruction).
                    jout = os.environ.get("KAN_JOUTER", "1") == "1"
                    pos = [psum_o.tile([P, O], f32, tag="po", name="po")
                           for _ in range(gu)]
                    drs = ((0, x2), (2, x2), (4, x3), (6, x3))
                    if jout:
                        f16_iter = [(k, j) for j in range(gu)
                                    for k in range(KS)]
                        dr_iter = [(d, j) for j in range(gu) for d in drs]
                    else:
                        f16_iter = [(k, j) for k in range(KS)
                                    for j in range(gu)]
                        dr_iter = [(d, j) for d in drs for j in range(gu)]
                    for k, j in f16_iter:
                        js = slice(j * P, (j + 1) * P)
                        nc.tensor.matmul(pos[j][:], xT[:, k, js],
                                         wa_sb[:, k, :],
                                         start=(k == 0), stop=False,
                                         skip_group_check=True)
                    for (i2, XT), j in dr_iter:
                        ks2 = slice(i2 % 4, i2 % 4 + 2)
                        js = slice(j * P, (j + 1) * P)
                        nc.tensor.matmul(pos[j][:], XT[:, ks2, js],
                                         w8_sb[:, i2:i2 + 2, :],
                                         perf_mode=DR,
                                         start=False, stop=(i2 == 6),
                                         skip_group_check=True)
                    for j in range(gu):
                        nc.vector.scalar_tensor_tensor(
                            o_sb[:, j, :], pos[j][:], 1.0 / SCALE, bsb[:],
                            mybir.AluOpType.mult, mybir.AluOpType.add)
                    if gu == 1:
                        nc.scalar.dma_start(o_1[t0], o_sb[:, 0, :])
                    else:
                        nc.scalar.dma_start(o_g[t0 // G], o_sb[:])
                    continue
                for j in range(gu):
                    js = slice(j * P, (j + 1) * P)
                    po = psum_o.tile([P, O], f32, tag="po", name="po")
                    if all8:
                        mms = [(xT, 0, wa_sb, 0), (xT, 2, wa_sb, 2),
                               (xT, 4, wa_sb, 0), (xT, 6, wa_sb, 2),
                               (xT, 0, wa_sb, 4), (xT, 2, wa_sb, 6),
                               (x2, 0, w8_sb, 0), (x2, 2, w8_sb, 2),
                               (x3, 0, w8_sb, 4), (x3, 2, w8_sb, 6)]
                        for i2, (XT, xk, WS, wk) in enumerate(mms):
                            nc.tensor.matmul(po[:], XT[:, xk:xk + 2, js],
                                             WS[:, wk:wk + 2, :],
                                             perf_mode=DR,
                                             start=(i2 == 0),
                                             stop=(i2 == len(mms) - 1),
                                             skip_group_check=True)
                    else:
                        for k in range(KS):
                            nc.tensor.matmul(po[:], xT[:, k, js],
                                             wa_sb[:, k, :],
                                             start=(k == 0), stop=False,
                                             skip_group_check=True)
                        if fp8:
                            for i2, XT in ((0, x2), (2, x2), (4, x3), (6, x3)):
                                ks2 = slice(i2 % 4, i2 % 4 + 2)
                                nc.tensor.matmul(po[:], XT[:, ks2, js],
                                                 w8_sb[:, i2:i2 + 2, :],
                                                 perf_mode=DR,
                                                 start=False, stop=(i2 == 6),
                                                 skip_group_check=True)
                        else:
                            for i2, XT in enumerate([x2] * 4 + [x3] * 4):
                                nc.tensor.matmul(po[:], XT[:, i2 % 4, js],
                                                 w8_sb[:, i2, :],
                                                 start=False, stop=(i2 == 7),
                                                 skip_group_check=True)
                    nc.vector.scalar_tensor_tensor(
                        o_sb[:, j, :], po[:], 1.0 / SCALE, bsb[:],
                        mybir.AluOpType.mult, mybir.AluOpType.add)
                oq = (nc.sync if gu == 1 and t0 in (N_TILES - 2, N_TILES - 4)
                      else nc.scalar)
                if gu == 1:
                    oq.dma_start(o_1[t0], o_sb[:, 0, :])
                else:
                    oq.dma_start(o_g[t0 // G], o_sb[:])

    nc.compile()
    return nc


_NC_CACHE: dict[tuple, bass.Bass] = {}


def _get_nc() -> bass.Bass:
    key = (_OUT_F32, _FP8)
    nc = _NC_CACHE.get(key)
    if nc is None:
        nc = _build()
        _NC_CACHE[key] = nc
    return nc


def _fold_weights(coeffs, W, b, alpha):
    import ml_dtypes

    a = 1.0 / (1.0 + np.exp(-np.float64(alpha)))
    s = a / np.sqrt(np.float64(I))
    A = (1.0 - a) * W.astype(np.float64).T + s * coeffs[:, :, 0].astype(np.float64)
    Bm = s * coeffs[:, :, 1].astype(np.float64)
    Cm = s * coeffs[:, :, 2].astype(np.float64)
    if _ALL8:
        asf = (A * SCALE).astype(np.float32)
        a_hi = asf.astype(ml_dtypes.float8_e4m3)
        a_lo = (asf - a_hi.astype(np.float32)).astype(ml_dtypes.float8_e4m3)
        wa = np.ascontiguousarray(np.concatenate([a_hi, a_lo], axis=0))
    else:
        wa = np.ascontiguousarray((A * SCALE).astype(np.float16))
    w8f = np.concatenate([Bm, Cm], axis=0) * (BSCALE * SCALE)
    if _FP8:
        w8 = np.ascontiguousarray(
            np.clip(w8f, -240.0, 240.0).astype(ml_dtypes.float8_e4m3))
    else:
        w8 = np.ascontiguousarray(w8f.astype(np.float16))
    b_eff = ((1.0 - a) * b.astype(np.float64)).astype(np.float32)
    bias_rep = np.ascontiguousarray(
        np.broadcast_to(b_eff[None, :], (P, O)).astype(np.float32))
    return wa, w8, bias_rep


def _make_in_maps(x, coeffs, W, b, alpha):
    wa, w8, bias_rep = _fold_weights(coeffs, W, b, alpha)
    x = np.asarray(x, dtype=np.float32)
    in_maps = []
    for c in range(N_CORES):
        shard = x[c * BS:(c + 1) * BS]
        if _ALL8:
            import ml_dtypes

            xt_f = shard.T.astype(np.float32)
            x_hi = xt_f.astype(ml_dtypes.float8_e4m3)
            x_lo = (xt_f - x_hi.astype(np.float32)).astype(
                ml_dtypes.float8_e4m3)
            xt_v = np.ascontiguousarray(np.concatenate([x_hi, x_lo], axis=0))
        else:
            xt_v = np.ascontiguousarray(shard.T.astype(np.float16))
        in_maps.append({
            "wa": wa, "w8": w8, "bias": bias_rep,
            "xt": xt_v,
        })
    return in_maps


def _run(x, coeffs, W, b, alpha, trace=False):
    nc = _get_nc()
    in_maps = _make_in_maps(x, coeffs, W, b, alpha)
    res = run_bass_kernel_spmd(nc, in_maps, core_ids=list(range(N_CORES)),
                               trace=trace)
    out = np.concatenate(
        [np.asarray(r["out"], dtype=np.float32) for r in res.results], axis=0)
    return out, res


def kernel(x, coeffs, W, b, alpha):
    out, _ = _run(x, coeffs, W, b, alpha, trace=False)
    return out
